# revision 25
# baseline (speedup 1.0000x reference)
"""Mixtral sparse-MoE block (T=8192, H=1024, I=3584, E=8, top-2) on 8 TRN2 cores.

Strategy: expert-parallel. The tiny gate (0.004% of FLOPs) runs on host in
fp64; tokens are dispatched (gathered) per expert on host, one expert per
core, padded to a common capacity C so all 8 cores run one SPMD program.
Each core computes the SwiGLU expert MLP over its compacted token batch:

    h = silu(x @ w1.T) * (x @ w3.T)        [C, I]
    y = combine_scale * (h @ w2.T)         [C, H]

and the host scatter-adds per-expert outputs back to [T, H].

Kernel (per core), matmuls in float32r (fp32 with 11-bit mantissa, full PE
rate at N>=256). Tokens processed in blocks of up to 1024 (8 tiles of 128):
  phase 1 (per 128-wide I-tile): psum[i,t] = sum_h w1t[h,i]*xT[h,t]  (I on
    partitions, tokens moving, N=512 chunks) -> silu/mul -> h tile resident.
  phase 2 (per 128-token tile): psum[t,hh] = sum_i h[i,t]*w2t[i,hh]  (tokens
    on partitions) -> scale by combine weight -> DMA out. All 8 token tiles
    of the block accumulate at once (8 PSUM banks: 6 + the 2 phase-1 banks),
    so w2 streams exactly once per block.
Weights stream from HBM with host-prepacked layouts; every DMA is >=512KB
contiguous-per-partition.
"""

import os
import sys
from contextlib import ExitStack

import numpy as np

for _p in ("/opt/trn_rl_repo", "/root/.axon_site/_ro/trn_rl_repo"):
    if os.path.isdir(_p) and _p not in sys.path:
        sys.path.insert(0, _p)
        break

T, H, I, E, TOPK = 8192, 1024, 3584, 8, 2
N_CORES = 8
P = 128
HC = H // P  # 8 contraction chunks of 128
IT = I // P  # 28 i-tiles

# matmul dtype: "bf16", "fp32r" (fp32 data @ full PE rate, 11-bit mantissa), "fp32"
MM_DTYPE = os.environ.get("MOE_MM_DTYPE", "fp32r")

_PROGRAM_CACHE: dict = {}


def _np_dt(dt_str):
    if dt_str == "bf16":
        import ml_dtypes

        return ml_dtypes.bfloat16
    return np.float32


def _round_fp32r(a):
    """Round fp32 -> fp32r (11-bit mantissa, RNE, low 12 bits zero) on host,
    matching walrus's fp32_to_fp32r. Assumes finite inputs."""
    u = np.ascontiguousarray(a, np.float32).view(np.uint32)
    lsb = (u >> 12) & 1
    u = (u + 0x7FF + lsb) & np.uint32(0xFFFFF000)
    return u.view(np.float32)


def _block_plan(nt):
    """Split NT 128-token tiles into blocks of 8 (1024 tokens) + remainder.

    Returns a list of ("normal", g0, gn) / ("merged", g0, 8, rem) /
    ("tiny", 0, nt) entries. A remainder of <=2 tiles is merged into the last
    full block so it shares that block's w1/w3 weight stream, with its
    phase 2 interleaved at lag 1.
    """
    full, rem = nt // 8, nt % 8
    if full == 0:
        return [("tiny", 0, nt)]
    plan = [("normal", 8 * i, 8) for i in range(full)]
    if rem == 0:
        return plan
    if rem <= 2:
        g0 = plan[-1][1]
        plan[-1] = ("merged", g0, 8, rem)
        return plan
    plan.append(("normal", 8 * full, rem))
    return plan


def _chunks(n, step):
    out, o = [], 0
    while o < n:
        out.append((o, min(step, n - o)))
        o += step
    return out


def _build_program(NT, dt_str):
    """Build + compile the SPMD Bass program for NT 128-token tiles."""
    import concourse.mybir as mybir
    import concourse.tile as tile
    from concourse import bacc

    key = (NT, dt_str)
    if key in _PROGRAM_CACHE:
        return _PROGRAM_CACHE[key]

    DT = {
        "bf16": mybir.dt.bfloat16,
        "fp32r": mybir.dt.float32r,
        "fp32": mybir.dt.float32,
    }[dt_str]
    f32 = mybir.dt.float32
    C = NT * P
    NHB = (C + 511) // 512  # x half-blocks of 512 tokens
    XW = NHB * 512

    nc = bacc.Bacc("TRN2", target_bir_lowering=False, debug=False, num_devices=N_CORES)
    xt = nc.dram_tensor("xt", [NHB, P, HC * 512], DT, kind="ExternalInput").ap()
    w1p = nc.dram_tensor("w1p", [IT, P, HC * P], DT, kind="ExternalInput").ap()
    w3p = nc.dram_tensor("w3p", [IT, P, HC * P], DT, kind="ExternalInput").ap()
    w2t = nc.dram_tensor("w2t", [I, H], DT, kind="ExternalInput").ap()
    sc = nc.dram_tensor("sc", [NT, P, 1], f32, kind="ExternalInput").ap()
    y = nc.dram_tensor("y", [C, H], f32, kind="ExternalOutput").ap()

    Silu = mybir.ActivationFunctionType.Silu
    Copy = mybir.ActivationFunctionType.Copy

    plan = _block_plan(NT)

    with tile.TileContext(nc) as tc:
        with ExitStack() as ctx:
            xpool = ctx.enter_context(tc.tile_pool(name="xb", bufs=2))
            wpool = ctx.enter_context(tc.tile_pool(name="w13", bufs=2))
            w2pool = ctx.enter_context(tc.tile_pool(name="w2", bufs=4))
            hpool = ctx.enter_context(tc.tile_pool(name="h", bufs=IT))
            tmppool = ctx.enter_context(tc.tile_pool(name="tmp", bufs=3))
            ypool = ctx.enter_context(tc.tile_pool(name="y", bufs=4))
            spool = ctx.enter_context(tc.tile_pool(name="s", bufs=12))
            ps1 = ctx.enter_context(tc.tile_pool(name="ps1", bufs=1, space="PSUM"))
            ps2 = ctx.enter_context(tc.tile_pool(name="ps2", bufs=1, space="PSUM"))

            def psum_ph(which):  # phase-1 psum banks, shared with phase-2 py6/py7
                return ps1.tile([P, 512], f32, tag=which, name=which)

            def load_w13(it):
                w1s = wpool.tile([P, HC, P], DT, tag="w1s", name="w1s")
                nc.sync.dma_start(
                    w1s[:], w1p[it].rearrange("p (c i) -> p c i", c=HC)
                )
                w3s = wpool.tile([P, HC, P], DT, tag="w3s", name="w3s")
                nc.sync.dma_start(
                    w3s[:], w3p[it].rearrange("p (c i) -> p c i", c=HC)
                )
                return w1s, w3s

            def p1_chunks(w1s, w3s, xhbs, chunk_list, ht):
                # one I-tile of phase 1: psum = w1/w3 against the token chunks
                for ci, (tc0, tcs) in chunk_list:
                    xhb = xhbs[ci]
                    ph1 = psum_ph("ph1")
                    ph3 = psum_ph("ph3")
                    for c in range(HC):
                        nc.tensor.matmul(
                            ph1[:, :tcs], w1s[:, c, :], xhb[:, c, :tcs],
                            start=(c == 0), stop=(c == HC - 1),
                        )
                    for c in range(HC):
                        nc.tensor.matmul(
                            ph3[:, :tcs], w3s[:, c, :], xhb[:, c, :tcs],
                            start=(c == 0), stop=(c == HC - 1),
                        )
                    sil = tmppool.tile([P, 512], f32, tag="sil", name="sil")
                    nc.scalar.activation(sil[:, :tcs], ph1[:, :tcs], Silu)
                    nc.vector.tensor_mul(
                        ht[:, tc0 : tc0 + tcs], sil[:, :tcs], ph3[:, :tcs]
                    )

            def load_xhbs(toff, tb, tag="xhb", shape=512, bufs=None, split_first=False):
                xhbs = []
                for j0, js in _chunks(tb, 512):
                    xhb = xpool.tile([P, HC, shape], DT, tag=tag, name=tag, bufs=bufs)
                    src = xt[(toff + j0) // 512].rearrange("p (c t) -> p c t", c=HC)
                    if split_first and j0 == 0:
                        # per-chunk loads so the first matmul group can start
                        # after 256KB instead of 2MB (kernel head latency)
                        for c in range(HC):
                            nc.sync.dma_start(
                                xhb[:, c : c + 1, :js], src[:, c : c + 1, :js]
                            )
                    else:
                        nc.sync.dma_start(xhb[:, :, :js], src[:, :, :js])
                    xhbs.append(xhb)
                return xhbs

            def load_scales(g0, gn):
                stiles = []
                for tl in range(gn):
                    st = spool.tile([P, 1], f32, tag="s", name="st")
                    nc.sync.dma_start(st[:], sc[g0 + tl, :, :])
                    stiles.append(st)
                return stiles

            def store_y(tglob, hh, py, st):
                yt = ypool.tile([P, 512], f32, tag="y", name="yt")
                nc.scalar.activation(yt[:], py[:], Copy, scale=st[:])
                nc.sync.dma_start(
                    y[tglob * P : (tglob + 1) * P, hh * 512 : (hh + 1) * 512], yt[:]
                )

            # PE warm-up under the initial DMA fill: ~5us of throwaway matmuls
            # trip the HAM activity window so the first real matmuls run at
            # 2.4GHz, and a 1-element Silu preloads the ACT table. Outputs are
            # never read.
            scr = tmppool.tile([P, 512], f32, tag="sil", name="scr")
            nc.any.memset(scr[:, :64], 0.0)
            nc.scalar.activation(scr[:, 64:65], scr[:, :1], Silu)
            pwarm = ps2.tile([P, 512], f32, tag="py0", name="pwarm")
            for _ in range(34):
                nc.tensor.matmul(
                    pwarm[:64, :64], scr[:, :64], scr[:, :64], start=True, stop=True
                )

            def phase2_block(g0, gn, hs, stiles):
                for hh in range(2):
                    pys = []
                    for tl in range(gn):
                        if tl < 6:
                            pys.append(
                                ps2.tile([P, 512], f32, tag=f"py{tl}", name=f"py{tl}")
                            )
                        else:  # borrow the phase-1 banks (idle during phase 2)
                            pys.append(psum_ph("ph1" if tl == 6 else "ph3"))
                    for icp in range(0, IT, 2):  # paired 512KB w2 loads
                        npair = min(2, IT - icp)
                        w2s = w2pool.tile([P, 2, 512], DT, tag="w2s", name="w2s")
                        nc.sync.dma_start(
                            w2s[:, :npair],
                            w2t[
                                icp * P : (icp + npair) * P,
                                hh * 512 : (hh + 1) * 512,
                            ].rearrange("(a p) n -> p a n", p=P),
                        )
                        for a in range(npair):
                            ic = icp + a
                            for tl in range(gn):
                                nc.tensor.matmul(
                                    pys[tl][:],
                                    hs[ic][:, tl * P : (tl + 1) * P],
                                    w2s[:, a, :],
                                    start=(ic == 0), stop=(ic == IT - 1),
                                )
                    # read the borrowed banks (6,7) first so the next block's
                    # phase 1 reclaims them promptly, then ascending so py0 —
                    # which the next hh pass's first matmul reuses — drains
                    # early instead of last
                    order = [t for t in (6, 7) if t < gn] + [
                        t for t in range(min(gn, 6))
                    ]
                    for tl in order:
                        store_y(g0 + tl, hh, pys[tl], stiles[tl])

            first = True
            for entry in plan:
                kind = entry[0]
                if kind == "normal":
                    _, g0, gn = entry
                    tb = gn * P
                    toff = g0 * P  # 512-aligned (all preceding blocks are 8)
                    if first:
                        # first weight tile before x so the PE can start on
                        # chunk 0 as soon as ~768KB have landed
                        w13_0 = load_w13(0)
                        xhbs = load_xhbs(toff, tb, split_first=True)
                    else:
                        xhbs = load_xhbs(toff, tb)
                    chunk_list = list(enumerate(_chunks(tb, 512)))
                    hs = []
                    for it in range(IT):
                        w1s, w3s = (
                            w13_0 if (first and it == 0) else load_w13(it)
                        )
                        ht = hpool.tile([P, tb], DT, tag="h", name="ht")
                        hs.append(ht)
                        p1_chunks(w1s, w3s, xhbs, chunk_list, ht)
                    first = False
                    stiles = load_scales(g0, gn)
                    phase2_block(g0, gn, hs, stiles)
                elif kind == "merged":
                    # last full block + <=2-tile tail sharing one w1/w3 stream;
                    # the tail's phase 2 interleaves at lag 1 inside phase 1
                    _, g0, gn, rem = entry
                    tb = gn * P
                    toff = g0 * P
                    tg0 = g0 + gn  # tail tile start
                    ttb = rem * P
                    xhbs = load_xhbs(toff, tb)
                    xtl = load_xhbs(tg0 * P, ttb, tag="xtl", shape=ttb, bufs=1)
                    chunk_list = list(enumerate(_chunks(tb, 512)))
                    tail_chunks = [(0, (0, ttb))]
                    stiles = load_scales(g0, gn)
                    stl = load_scales(tg0, rem)
                    pys_t = [
                        [
                            ps2.tile(
                                [P, 512], f32,
                                tag=f"py{2 * tl + hh}", name=f"py{2 * tl + hh}",
                            )
                            for hh in range(2)
                        ]
                        for tl in range(rem)
                    ]
                    hs = []
                    hts = []
                    for it in range(IT + 1):
                        if it < IT:
                            w1s, w3s = load_w13(it)
                            ht = hpool.tile([P, tb], DT, tag="h", name="ht")
                            hs.append(ht)
                            p1_chunks(w1s, w3s, xhbs, chunk_list, ht)
                            htl = hpool.tile([P, ttb], DT, tag="htl", bufs=3, name="htl")
                            hts.append(htl)
                            p1_chunks(w1s, w3s, xtl, tail_chunks, htl)
                        if it >= 1:
                            ic = it - 1
                            w2f = w2pool.tile([P, H], DT, tag="w2s", name="w2f")
                            nc.sync.dma_start(w2f[:], w2t[ic * P : (ic + 1) * P, :])
                            for hh in range(2):
                                for tl in range(rem):
                                    nc.tensor.matmul(
                                        pys_t[tl][hh][:],
                                        hts[ic][:, tl * P : (tl + 1) * P],
                                        w2f[:, hh * 512 : (hh + 1) * 512],
                                        start=(ic == 0), stop=(ic == IT - 1),
                                    )
                    for tl in range(rem):
                        for hh in range(2):
                            store_y(tg0 + tl, hh, pys_t[tl][hh], stl[tl])
                    phase2_block(g0, gn, hs, stiles)
                else:  # tiny: NT <= 2, interleaved standalone
                    _, g0, gn = entry
                    tb = gn * P
                    xhbs = load_xhbs(0, tb, tag="xhb")
                    stiles = load_scales(g0, gn)
                    hs = []
                    pys = [
                        [
                            ps2.tile(
                                [P, 512], f32,
                                tag=f"py{2 * tl + hh}", name=f"py{2 * tl + hh}",
                            )
                            for hh in range(2)
                        ]
                        for tl in range(gn)
                    ]
                    for it in range(IT + 1):
                        if it < IT:
                            w1s, w3s = load_w13(it)
                            ht = hpool.tile([P, tb], DT, tag="h", name="ht")
                            hs.append(ht)
                            p1_chunks(w1s, w3s, xhbs, [(0, (0, tb))], ht)
                        if it >= 1:
                            ic = it - 1
                            w2f = w2pool.tile([P, H], DT, tag="w2s", name="w2f")
                            nc.sync.dma_start(w2f[:], w2t[ic * P : (ic + 1) * P, :])
                            for hh in range(2):
                                for tl in range(gn):
                                    nc.tensor.matmul(
                                        pys[tl][hh][:],
                                        hs[ic][:, tl * P : (tl + 1) * P],
                                        w2f[:, hh * 512 : (hh + 1) * 512],
                                        start=(ic == 0), stop=(ic == IT - 1),
                                    )
                    for tl in range(gn):
                        for hh in range(2):
                            store_y(g0 + tl, hh, pys[tl][hh], stiles[tl])

    nc.compile()
    _PROGRAM_CACHE[key] = nc
    return nc


def _route(x, gate_w, gate_b):
    """Top-2 routing on host, fp64 (verified to match the jax fp32 reference)."""
    lg = x.astype(np.float64) @ gate_w.T.astype(np.float64) + gate_b.astype(np.float64)
    lg -= lg.max(axis=-1, keepdims=True)
    p = np.exp(lg)
    p /= p.sum(axis=-1, keepdims=True)
    i1 = np.argmax(p, axis=-1)
    v1 = p[np.arange(p.shape[0]), i1]
    p2 = p.copy()
    p2[np.arange(p.shape[0]), i1] = -1.0
    i2 = np.argmax(p2, axis=-1)
    v2 = p2[np.arange(p2.shape[0]), i2]
    return (
        np.stack([i1, i2], 1),
        np.stack([v1, v2], 1).astype(np.float32),
    )


def _run_spmd(nc, in_maps, profile=False):
    from concourse import bass_utils

    core_ids = list(range(N_CORES))
    # First execution after NEFF load has shown sporadic stale-memory reads;
    # warm up, then run until two consecutive executions agree bitwise.
    bass_utils.run_bass_kernel_spmd(nc, in_maps, core_ids=core_ids)
    prev = None
    res = None
    for _ in range(4):
        res = bass_utils.run_bass_kernel_spmd(nc, in_maps, core_ids=core_ids)
        cur = [r["y"] for r in res.results]
        if prev is not None and all(
            np.array_equal(a, b) for a, b in zip(prev, cur)
        ):
            break
        prev = cur
    exec_ns = None
    if profile:
        pres = bass_utils.run_bass_kernel_spmd(
            nc, in_maps, core_ids=core_ids, trace=True
        )
        exec_ns = pres.exec_time_ns
    return res, exec_ns


def run(inputs, profile=False, dt_str=None):
    dt_str = dt_str or MM_DTYPE
    x = np.ascontiguousarray(np.asarray(inputs["x"], np.float32))
    gate_w = np.asarray(inputs["gate_w"], np.float32)
    gate_b = np.asarray(inputs["gate_b"], np.float32)
    w1 = np.asarray(inputs["w1"], np.float32)
    w2 = np.asarray(inputs["w2"], np.float32)
    w3 = np.asarray(inputs["w3"], np.float32)

    idx, val = _route(x, gate_w, gate_b)

    toks = []  # per-expert token index lists
    svals = []
    for e in range(E):
        m = idx == e  # [T, 2]
        sel = m.any(axis=1)
        te = np.nonzero(sel)[0]
        se = np.where(m[te, 0], val[te, 0], val[te, 1])
        toks.append(te)
        svals.append(se.astype(np.float32))
    maxcnt = max(len(t) for t in toks)

    # capacity: NT 128-token tiles; cap bounded by SBUF h-residency (block<=8)
    cap_tiles = 36 if dt_str == "bf16" else 18
    NT = min(max((maxcnt + P - 1) // P, 1), cap_tiles)
    CR = NT * P
    n_runs = (maxcnt + CR - 1) // CR

    npdt = _np_dt(dt_str)
    xT = np.ascontiguousarray(x.T)  # [H, T] fp32

    nc = _build_program(NT, dt_str)
    NHB = (CR + 511) // 512
    XW = NHB * 512

    def conv(a):
        a = np.ascontiguousarray(a)
        if dt_str == "fp32r":
            return _round_fp32r(a)
        return a.astype(npdt)

    wmaps = []
    for e in range(E):
        # w1p/w3p: [IT, P, HC*P] with w1p[it, p, c*P+ii] = w1[e].T[c*P+p, it*P+ii]
        w1pk = (
            w1[e].T.reshape(HC, P, IT, P).transpose(2, 1, 0, 3).reshape(IT, P, HC * P)
        )
        w3pk = (
            w3[e].T.reshape(HC, P, IT, P).transpose(2, 1, 0, 3).reshape(IT, P, HC * P)
        )
        wmaps.append(dict(w1p=conv(w1pk), w3p=conv(w3pk), w2t=conv(w2[e].T)))

    out = np.zeros((T, H), np.float32)
    exec_ns = None
    for r in range(n_runs):
        in_maps = []
        seg_toks = []
        for e in range(E):
            te = toks[e][r * CR : (r + 1) * CR]
            se = svals[e][r * CR : (r + 1) * CR]
            seg_toks.append(te)
            cnt = len(te)
            xg = np.zeros((H, XW), np.float32)
            if cnt:
                xg[:, :cnt] = xT[:, te]
            # half-block-packed: [NHB, P, HC*512], xt[j,p,c*512+t]=xg[c*P+p, j*512+t]
            xg = (
                xg.reshape(HC, P, NHB, 512)
                .transpose(2, 1, 0, 3)
                .reshape(NHB, P, HC * 512)
            )
            scp = np.zeros(CR, np.float32)
            scp[:cnt] = se
            in_maps.append(
                dict(xt=conv(xg), sc=scp.reshape(NT, P, 1), **wmaps[e])
            )
        res, ens = _run_spmd(nc, in_maps, profile=profile and r == 0)
        if ens is not None:
            exec_ns = ens
        for e in range(E):
            te = seg_toks[e]
            if len(te):
                out[te] += res.results[e]["y"][: len(te)]
    return out, exec_ns


def kernel(**inputs):
    out, _ = run(inputs, profile=False)
    return out
